# revision 1
# baseline (speedup 1.0000x reference)
"""GAT message-passing kernel for Trainium2 (8 NeuronCores, SPMD).

Strategy: shard edges by TARGET node range (each core owns NLOC=6272 of the
padded 50176 nodes and all edges targeting them). Per core, targets are
grouped into 49 tiles of 128 nodes; each tile's edges are processed in
chunks of 128:
  - per-edge x[src], sj[src]: batched dma_gather (int16 idx; lo/hi arena
    split at 32768 source rows)
  - per-edge si[tgt], recip[tgt]: batched dma_gather from a core-local
    [si|recip] table (tgt_local < 6272 fits int16 directly)
  - denominator segment-sum and output scatter-add: one-hot matmuls
    (one-hot built by DVE iota==tgt_off compare) accumulated in PSUM
Score tables si/sj are computed once from each core's node slice; sj is
AllGather'd. No other collectives are needed.
"""
import numpy as np

import concourse.mybir as mybir
from concourse import bacc, bass_utils
from concourse.tile import TileContext

P = 128
NCORES = 8
N_NODES = 50000
N_EDGES = 800000
HID = 128
HEADS = 8
NPAD = 50176              # 8 * 6272
NLOC = NPAD // NCORES     # 6272 nodes per core
NT = NLOC // P            # 49 tiles per core
SPLIT = 32768             # lo/hi arena split for int16 dma_gather indices
NEG_SLOPE = 0.01
SJW = 64                  # sj table row width (f32) -> 256B rows for dma_gather
SRW = 64                  # [si|recip] table row width
GMAX = 8                  # slots per dma_gather call (1024 idx HW limit)

_CACHE = {}


def _build_program(nclo, nchi, batches):
    nch = [lo + hi for lo, hi in zip(nclo, nchi)]
    nchunks = sum(nch)
    nslot_lo = sum(nclo) * P
    nslot_hi = sum(nchi) * P
    lo_base = np.cumsum([0] + nclo).tolist()
    hi_base = np.cumsum([0] + nchi).tolist()
    ch_base = np.cumsum([0] + nch).tolist()

    nc = bacc.Bacc("TRN2", num_devices=NCORES)
    f32 = mybir.dt.float32

    xpad = nc.dram_tensor("xpad", [NPAD, HID], f32, kind="ExternalInput")
    xslice = nc.dram_tensor("xslice", [NLOC, HID], f32, kind="ExternalInput")
    wcat = nc.dram_tensor("wcat", [HID, 2 * HEADS], f32, kind="ExternalInput")
    idxlo = nc.dram_tensor("idxlo", [P, max(nslot_lo // 16, 1)],
                           mybir.dt.int16, kind="ExternalInput")
    idxhi = nc.dram_tensor("idxhi", [P, max(nslot_hi // 16, 1)],
                           mybir.dt.int16, kind="ExternalInput")
    idxtg = nc.dram_tensor("idxtg", [P, nchunks * 8], mybir.dt.int16,
                           kind="ExternalInput")
    toffin = nc.dram_tensor("toffin", [P, nchunks], f32, kind="ExternalInput")
    out_sl = nc.dram_tensor("out_sl", [NLOC, HID], f32, kind="ExternalOutput")

    sjtab = nc.dram_tensor("sjtab", [NPAD, SJW], f32, kind="Internal")
    sitab = nc.dram_tensor("sitab", [NLOC, SRW], f32, kind="Internal")
    retab = nc.dram_tensor("retab", [NLOC, SRW], f32, kind="Internal")
    cc_in = nc.dram_tensor("cc_in", [NLOC, HEADS], f32, kind="Internal")
    cc_out = nc.dram_tensor("cc_out", [NPAD, HEADS], f32, kind="Internal",
                            addr_space="Shared")

    ident_d = nc.inline_tensor(np.eye(P, dtype=np.float32), name="identc")
    iota_d = nc.inline_tensor(
        np.tile(np.arange(P, dtype=np.float32), (P, 1)), name="iotac")
    zero_d = nc.inline_tensor(np.zeros((P, SJW), np.float32), name="zeroc")

    with TileContext(nc) as tc:
        with tc.tile_pool(name="const", bufs=1) as constp, \
             tc.tile_pool(name="ph0", bufs=3) as ph0:

            ident = constp.tile([P, P], f32)
            nc.sync.dma_start(out=ident[:], in_=ident_d[:, :])
            iota_f = constp.tile([P, P], f32)
            nc.sync.dma_start(out=iota_f[:], in_=iota_d[:, :])
            wc_sb = constp.tile([HID, 2 * HEADS], f32)
            nc.sync.dma_start(out=wc_sb[:], in_=wcat[:, :])
            toff_sb = constp.tile([P, nchunks], f32)
            nc.sync.dma_start(out=toff_sb[:], in_=toffin[:, :])
            ixlo_sb = constp.tile([P, max(nslot_lo // 16, 1)], mybir.dt.int16)
            nc.sync.dma_start(out=ixlo_sb[:], in_=idxlo[:, :])
            ixhi_sb = constp.tile([P, max(nslot_hi // 16, 1)], mybir.dt.int16)
            nc.sync.dma_start(out=ixhi_sb[:], in_=idxhi[:, :])
            ixtg_sb = constp.tile([P, nchunks * 8], mybir.dt.int16)
            nc.sync.dma_start(out=ixtg_sb[:], in_=idxtg[:, :])
            zt = constp.tile([P, 1, SJW], f32)
            nc.sync.dma_start(out=zt[:, 0, :], in_=zero_d[:, :])

            # zero pad columns of both gather tables (gathers read whole
            # 256B rows; sim rejects uninitialized reads)
            nc.sync.dma_start(
                out=sjtab[:, HEADS:SJW].rearrange("(t p) w -> p t w", p=P),
                in_=zt[:, :, 0:SJW - HEADS].to_broadcast(
                    [P, NPAD // P, SJW - HEADS]))
            nc.sync.dma_start(
                out=sitab[:, HEADS:SRW].rearrange("(t p) w -> p t w", p=P),
                in_=zt[:, :, 0:SRW - HEADS].to_broadcast(
                    [P, NT, SRW - HEADS]))
            nc.sync.dma_start(
                out=retab[:, 0:SRW].rearrange("(t p) w -> p t w", p=P),
                in_=zt[:, :, 0:SRW].to_broadcast([P, NT, SRW]))

            # ---------- phase 0: score tables ----------
            with tc.tile_pool(name="ph0ps", bufs=2, space="PSUM") as ph0ps:
                for j in range(NT):
                    xt = ph0.tile([P, HID], f32, tag="xt")
                    nc.sync.dma_start(out=xt[:], in_=xslice[j * P:(j + 1) * P, :])
                    xT_ps = ph0ps.tile([P, P], f32, space="PSUM", tag="xTp")
                    nc.tensor.transpose(out=xT_ps[:], in_=xt[:], identity=ident[:])
                    xT = ph0.tile([P, P], f32, tag="xT")
                    nc.scalar.copy(out=xT[:], in_=xT_ps[:])
                    sc_ps = ph0ps.tile([P, 2 * HEADS], f32, space="PSUM", tag="scp")
                    nc.tensor.matmul(out=sc_ps[:], lhsT=xT[:], rhs=wc_sb[:],
                                     start=True, stop=True)
                    sc = ph0.tile([P, 2 * HEADS], f32, tag="sc")
                    nc.vector.tensor_copy(out=sc[:], in_=sc_ps[:])
                    nc.sync.dma_start(out=sitab[j * P:(j + 1) * P, 0:HEADS],
                                      in_=sc[:, 0:HEADS])
                    nc.sync.dma_start(out=cc_in[j * P:(j + 1) * P, :],
                                      in_=sc[:, HEADS:2 * HEADS])

            nc.gpsimd.collective_compute(
                "AllGather", mybir.AluOpType.bypass,
                replica_groups=[list(range(NCORES))],
                ins=[cc_in[:, :]], outs=[cc_out[:, :]],
            )
            nc.sync.dma_start(
                out=sjtab[:, 0:HEADS].rearrange("(t p) w -> p t w", p=P),
                in_=cc_out[:, :].rearrange("(t p) w -> p t w", p=P))

            with tc.tile_pool(name="gat", bufs=2) as gatp, \
                 tc.tile_pool(name="oh", bufs=5) as ohp, \
                 tc.tile_pool(name="tile", bufs=4) as tilep, \
                 tc.tile_pool(name="sm", bufs=4) as smp, \
                 tc.tile_pool(name="ps_den", bufs=4, space="PSUM") as psd, \
                 tc.tile_pool(name="ps_out", bufs=4, space="PSUM") as pso:

                def gcalls(dst, table_ap, idx_sb, s0, s1):
                    ew = dst.shape[-1]
                    for g0 in range(0, s1 - s0, GMAX):
                        g1 = min(g0 + GMAX, s1 - s0)
                        nidx = (g1 - g0) * P
                        nc.gpsimd.dma_gather(
                            out_ap=dst[:, g0:g1, :], in_ap=table_ap,
                            idxs_ap=idx_sb[:, (s0 + g0) * 8:(s0 + g1) * 8],
                            num_idxs=nidx, num_idxs_reg=nidx, elem_size=ew)

                for (t0, t1) in batches:
                    blo0, blo1 = lo_base[t0], lo_base[t1]
                    bhi0, bhi1 = hi_base[t0], hi_base[t1]
                    bch0, bch1 = ch_base[t0], ch_base[t1]

                    gx_lo = gatp.tile([P, max(blo1 - blo0, 1), HID], f32,
                                      tag="gxlo")
                    gs_lo = gatp.tile([P, max(blo1 - blo0, 1), SJW], f32,
                                      tag="gslo")
                    if blo1 > blo0:
                        gcalls(gx_lo[:], xpad[:, :], ixlo_sb, blo0, blo1)
                        gcalls(gs_lo[:], sjtab[:, :], ixlo_sb, blo0, blo1)
                    gx_hi = gatp.tile([P, max(bhi1 - bhi0, 1), HID], f32,
                                      tag="gxhi")
                    gs_hi = gatp.tile([P, max(bhi1 - bhi0, 1), SJW], f32,
                                      tag="gshi")
                    if bhi1 > bhi0:
                        gcalls(gx_hi[:], xpad[SPLIT:NPAD, :], ixhi_sb, bhi0, bhi1)
                        gcalls(gs_hi[:], sjtab[SPLIT:NPAD, :], ixhi_sb, bhi0, bhi1)
                    # si gather (by tgt_local), valid cols 0:8
                    gsa = gatp.tile([P, bch1 - bch0, SRW], f32, tag="gsa")
                    gcalls(gsa[:], sitab[:, :], ixtg_sb, bch0, bch1)

                    # ---------- phase A per tile ----------
                    tile_state = []
                    for j in range(t0, t1):
                        ncj = nch[j]
                        nlo_j, nhi_j = nclo[j], nchi[j]
                        ch0 = ch_base[j]
                        ohs = []
                        ex = tilep.tile([P, ncj * HEADS], f32, tag="ex")
                        co = ch0 - bch0
                        if nlo_j:
                            s0 = lo_base[j] - blo0
                            nc.vector.tensor_tensor(
                                out=ex[:, 0:nlo_j * HEADS].rearrange(
                                    "p (k w) -> p k w", k=nlo_j),
                                in0=gsa[:, co:co + nlo_j, 0:HEADS],
                                in1=gs_lo[:, s0:s0 + nlo_j, 0:HEADS],
                                op=mybir.AluOpType.add)
                        if nhi_j:
                            s0 = hi_base[j] - bhi0
                            nc.vector.tensor_tensor(
                                out=ex[:, nlo_j * HEADS:ncj * HEADS].rearrange(
                                    "p (k w) -> p k w", k=nhi_j),
                                in0=gsa[:, co + nlo_j:co + ncj, 0:HEADS],
                                in1=gs_hi[:, s0:s0 + nhi_j, 0:HEADS],
                                op=mybir.AluOpType.add)
                        lk = tilep.tile([P, ncj * HEADS], f32, tag="lk")
                        nc.vector.tensor_scalar(
                            out=lk[:], in0=ex[:], scalar1=NEG_SLOPE,
                            scalar2=None, op0=mybir.AluOpType.mult)
                        nc.vector.tensor_tensor(out=ex[:], in0=ex[:], in1=lk[:],
                                                op=mybir.AluOpType.max)
                        nc.scalar.activation(
                            out=ex[:], in_=ex[:],
                            func=mybir.ActivationFunctionType.Exp)
                        den_ps = psd.tile([P, HEADS], f32, space="PSUM",
                                          tag="denps")
                        for c in range(ncj):
                            oh = ohp.tile([P, P], f32, tag=f"oh{c}")
                            nc.vector.tensor_scalar(
                                out=oh[:], in0=iota_f[:],
                                scalar1=toff_sb[:, ch0 + c:ch0 + c + 1],
                                scalar2=None, op0=mybir.AluOpType.is_equal)
                            nc.tensor.matmul(
                                out=den_ps[:], lhsT=oh[:],
                                rhs=ex[:, c * HEADS:(c + 1) * HEADS],
                                start=(c == 0), stop=(c == ncj - 1))
                            ohs.append(oh)
                        rec = smp.tile([P, HEADS], f32, tag="rec")
                        nc.vector.tensor_scalar(
                            out=rec[:], in0=den_ps[:], scalar1=1e-30,
                            scalar2=None, op0=mybir.AluOpType.max)
                        nc.vector.reciprocal(out=rec[:], in_=rec[:])
                        nc.vector.tensor_scalar(
                            out=rec[:], in0=rec[:], scalar1=1.0 / HEADS,
                            scalar2=None, op0=mybir.AluOpType.mult)
                        nc.sync.dma_start(
                            out=retab[j * P:(j + 1) * P, 0:HEADS],
                            in_=rec[:])
                        tile_state.append((j, ex, ohs))

                    # recip gather for the whole batch (rows now updated)
                    gsb = gatp.tile([P, bch1 - bch0, SRW], f32, tag="gsb")
                    gcalls(gsb[:], retab[:, :], ixtg_sb, bch0, bch1)

                    # ---------- phase B per tile ----------
                    for (j, ex, ohs) in tile_state:
                        ncj = nch[j]
                        nlo_j = nclo[j]
                        ch0 = ch_base[j]
                        co = ch0 - bch0
                        prod = smp.tile([P, ncj * HEADS], f32, tag="prod")
                        nc.vector.tensor_tensor(
                            out=prod[:].rearrange("p (k w) -> p k w", k=ncj),
                            in0=gsb[:, co:co + ncj, 0:HEADS],
                            in1=ex[:].rearrange("p (k w) -> p k w", k=ncj),
                            op=mybir.AluOpType.mult)
                        alpha = smp.tile([P, ncj], f32, tag="alpha")
                        nc.vector.reduce_sum(
                            out=alpha[:],
                            in_=prod[:].rearrange("p (k w) -> p k w", k=ncj),
                            axis=mybir.AxisListType.X)
                        out_ps = pso.tile([P, HID], f32, space="PSUM",
                                          tag="outps")
                        for c in range(ncj):
                            if c < nlo_j:
                                gx_ap = gx_lo[:, lo_base[j] - blo0 + c, :]
                            else:
                                gx_ap = gx_hi[:, hi_base[j] - bhi0 + (c - nlo_j), :]
                            oha = ohs[c]
                            nc.vector.tensor_scalar(
                                out=oha[:], in0=oha[:],
                                scalar1=alpha[:, c:c + 1],
                                scalar2=None, op0=mybir.AluOpType.mult)
                            nc.tensor.matmul(out=out_ps[:], lhsT=oha[:],
                                             rhs=gx_ap,
                                             start=(c == 0), stop=(c == ncj - 1))
                        ot = smp.tile([P, HID], f32, tag="ot")
                        nc.scalar.copy(out=ot[:], in_=out_ps[:])
                        nc.sync.dma_start(out=out_sl[j * P:(j + 1) * P, :],
                                          in_=ot[:])

    nc.compile()
    return nc


def _prep(edge_index):
    """Host-side edge layout -> per-core index/toff arrays + chunk schedule."""
    src = edge_index[0].astype(np.int64)
    tgt = edge_index[1].astype(np.int64)
    core = tgt // NLOC
    tile = (tgt % NLOC) // P
    toff = tgt % P
    tloc = tgt % NLOC
    lo = src < SPLIT

    counts = np.zeros((NCORES, NT, 2), np.int64)
    np.add.at(counts, (core, tile, (~lo).astype(np.int64)), 1)
    nclo = [int(np.ceil(max(counts[:, j, 0].max(), 1) / P)) for j in range(NT)]
    nchi = [int(np.ceil(counts[:, j, 1].max() / P))
            if counts[:, j, 1].max() > 0 else 0 for j in range(NT)]

    nch = [a + b for a, b in zip(nclo, nchi)]
    nchunks = sum(nch)
    nslot_lo = sum(nclo) * P
    nslot_hi = sum(nchi) * P
    lo_base = np.cumsum([0] + nclo)
    hi_base = np.cumsum([0] + nchi)
    ch_base = np.cumsum([0] + nch)

    per_core = []
    order = np.lexsort((tile, core))
    src_s, tile_s, toff_s, lo_s, core_s, tloc_s = (
        src[order], tile[order], toff[order], lo[order], core[order],
        tgt[order] % NLOC)
    cuts = np.searchsorted(core_s, np.arange(NCORES + 1))

    def wrap16(a):
        if len(a) == 0:
            return np.zeros((P, 1), np.int16)
        w = a.reshape(-1, 16).T
        return np.tile(w, (8, 1)).astype(np.int16)

    for c in range(NCORES):
        s, e = cuts[c], cuts[c + 1]
        csrc, ctile, ctoff, clo, ctloc = (src_s[s:e], tile_s[s:e],
                                          toff_s[s:e], lo_s[s:e], tloc_s[s:e])
        ilo = np.zeros(nslot_lo, np.int16)
        ihi = np.zeros(nslot_hi, np.int16)
        itg = np.zeros(nchunks * P, np.int16)
        tof = np.full(nchunks * P, 999.0, np.float32)
        tcuts = np.searchsorted(ctile, np.arange(NT + 1))
        for j in range(NT):
            js, je = tcuts[j], tcuts[j + 1]
            jsrc, jtoff, jlo, jtloc = (csrc[js:je], ctoff[js:je], clo[js:je],
                                       ctloc[js:je])
            sel = jlo
            n = int(sel.sum())
            ilo[lo_base[j] * P:lo_base[j] * P + n] = jsrc[sel].astype(np.int16)
            cb = ch_base[j] * P
            tof[cb:cb + n] = jtoff[sel]
            itg[cb:cb + n] = jtloc[sel].astype(np.int16)
            sel = ~jlo
            m = int(sel.sum())
            ihi[hi_base[j] * P:hi_base[j] * P + m] = \
                (jsrc[sel] - SPLIT).astype(np.int16)
            cb2 = (ch_base[j] + nclo[j]) * P
            tof[cb2:cb2 + m] = jtoff[sel]
            itg[cb2:cb2 + m] = jtloc[sel].astype(np.int16)

        per_core.append({
            "idxlo": wrap16(ilo),
            "idxhi": wrap16(ihi),
            "idxtg": wrap16(itg),
            "toffin": np.ascontiguousarray(
                tof.reshape(nchunks, P).T).astype(np.float32),
        })
    return nclo, nchi, per_core


def _in_maps(inputs, per_core):
    xpad = np.zeros((NPAD, HID), np.float32)
    xpad[:N_NODES] = inputs["x"]
    wcat = np.concatenate([np.asarray(inputs["Wi"]).T,
                           np.asarray(inputs["Wj"]).T],
                          axis=1).astype(np.float32)
    maps = []
    for c in range(NCORES):
        m = dict(per_core[c])
        m["xpad"] = xpad
        m["xslice"] = np.ascontiguousarray(xpad[c * NLOC:(c + 1) * NLOC])
        m["wcat"] = wcat
        maps.append(m)
    return maps


def kernel(x, Wi, Wj, edge_index):
    inputs = {"x": np.asarray(x, np.float32),
              "Wi": np.asarray(Wi, np.float32),
              "Wj": np.asarray(Wj, np.float32)}
    edge_index = np.asarray(edge_index)

    nclo, nchi, per_core = _prep(edge_index)
    key = (tuple(nclo), tuple(nchi))
    if key not in _CACHE:
        batches = [(t, min(t + 2, NT)) for t in range(0, NT, 2)]
        _CACHE.clear()
        _CACHE[key] = _build_program(nclo, nchi, batches)
    nc = _CACHE[key]

    res = bass_utils.run_bass_kernel_spmd(nc, _in_maps(inputs, per_core),
                                          core_ids=list(range(NCORES)))
    out = np.concatenate([res.results[c]["out_sl"] for c in range(NCORES)],
                         axis=0)
    return np.ascontiguousarray(out[:N_NODES])



# revision 8
# speedup vs baseline: 1.8248x; 1.8248x over previous
"""GAT message-passing kernel for Trainium2 (8 NeuronCores, SPMD).

Target-sharded edge processing, one packed dma_gather per edge:
  - host packs [x | sj] bf16 into 512B rows of one gather table; per-edge
    si[tgt] is uploaded as a contiguous slab (chunk layout), so each edge
    costs exactly ONE 512B gather descriptor (vs 4 in the old design).
  - per-chunk one-hot (DVE is_equal, bf16/4x) drives three matmuls:
    den[t,h] += oh^T ex, rec_edge = ohT^T rec, out[t,:] += oh^T (alpha*x).
    ohT comes from a PE transpose + Act-engine PSUM->SBUF copy.
  - leaky_relu/exp/softmax-normalize all on device (DVE stt + Act exp).
  - all heavy matmuls in bf16 (1 cy/row vs 4 for f32).
No collectives: each core owns a contiguous target range and all edges
pointing into it.
"""
import numpy as np
import ml_dtypes

import concourse.mybir as mybir
from concourse import bacc, bass_utils
from concourse.tile import TileContext

BF16 = ml_dtypes.bfloat16

P = 128
NCORES = 8
N_NODES = 50000
N_EDGES = 800000
HID = 128
HEADS = 8
NPAD = 50176              # 8 * 6272
NLOC = NPAD // NCORES     # 6272 targets per core
NT = NLOC // P            # 49 tiles per core
SPLIT = 32768             # lo/hi arena split (int16 gather indices)
NEG_SLOPE = 0.01
ROWW = 256                # packed row width in bf16 elems (512B)
GT = 3                    # tiles per gather group
GMAX = 8                  # slots per dma_gather call (1024-idx HW limit)
B = 8                     # chunks per ohT/exp sub-batch

_CACHE = {}


def _build_program(nclo, nchi):
    f32 = mybir.dt.float32
    bf16 = mybir.dt.bfloat16
    nch = [a + b for a, b in zip(nclo, nchi)]
    ngrp = (NT + GT - 1) // GT
    groups = [list(range(g * GT, min(g * GT + GT, NT))) for g in range(ngrp)]
    # global chunk ids: per group: [lo(j0) lo(j1) .. | hi(j0) hi(j1) ..]
    glo_off = {}
    ghi_off = {}
    grp_base = []
    cid = 0
    for g, tiles in enumerate(groups):
        grp_base.append(cid)
        for j in tiles:
            glo_off[j] = cid
            cid += nclo[j]
        for j in tiles:
            ghi_off[j] = cid
            cid += nchi[j]
    nchunks = cid
    lo_base = np.cumsum([0] + nclo).tolist()   # slot base in lo arena
    hi_base = np.cumsum([0] + nchi).tolist()

    nslot_lo = sum(nclo) * P
    nslot_hi = max(sum(nchi) * P, 16)

    nc = bacc.Bacc("TRN2", num_devices=NCORES)

    xsj = nc.dram_tensor("xsj", [NPAD, ROWW], bf16, kind="ExternalInput")
    idxlo = nc.dram_tensor("idxlo", [P, nslot_lo // 16], mybir.dt.int16,
                           kind="ExternalInput")
    idxhi = nc.dram_tensor("idxhi", [P, nslot_hi // 16], mybir.dt.int16,
                           kind="ExternalInput")
    toffin = nc.dram_tensor("toffin", [P, nchunks], f32, kind="ExternalInput")
    zprein = nc.dram_tensor("zprein", [P, nchunks * HEADS], bf16,
                            kind="ExternalInput")
    out_sl = nc.dram_tensor("out_sl", [NLOC, HID], f32, kind="ExternalOutput")

    iota_d = nc.inline_tensor(
        np.tile(np.arange(P, dtype=BF16), (P, 1)), name="iotac")
    ident_d = nc.inline_tensor(np.eye(P, dtype=BF16), name="identc")

    with TileContext(nc) as tc:
        lp = nc.allow_low_precision(reason="bf16 one-hot transpose, no accum")
        lp.__enter__()
        with tc.tile_pool(name="const", bufs=1) as constp, \
             tc.tile_pool(name="gat", bufs=2) as gatp, \
             tc.tile_pool(name="oh", bufs=2) as ohp, \
             tc.tile_pool(name="sm", bufs=2) as smp, \
             tc.tile_pool(name="ps_oht", bufs=2, space="PSUM") as psoht, \
             tc.tile_pool(name="ps_den", bufs=2, space="PSUM") as psden, \
             tc.tile_pool(name="ps_rec", bufs=2, space="PSUM") as psrec, \
             tc.tile_pool(name="ps_out", bufs=2, space="PSUM") as psout:

            iota_bf = constp.tile([P, P], bf16)
            nc.sync.dma_start(out=iota_bf[:], in_=iota_d[:, :])
            ident_bf = constp.tile([P, P], bf16)
            nc.sync.dma_start(out=ident_bf[:], in_=ident_d[:, :])
            toff_sb = constp.tile([P, nchunks], f32)
            nc.sync.dma_start(out=toff_sb[:], in_=toffin[:, :])
            zpre_sb = constp.tile([P, nchunks, HEADS], bf16)
            nc.sync.dma_start(
                out=zpre_sb[:].rearrange("p c h -> p (c h)"),
                in_=zprein[:, :])
            ixlo_sb = constp.tile([P, nslot_lo // 16], mybir.dt.int16)
            nc.sync.dma_start(out=ixlo_sb[:], in_=idxlo[:, :])
            ixhi_sb = constp.tile([P, nslot_hi // 16], mybir.dt.int16)
            nc.sync.dma_start(out=ixhi_sb[:], in_=idxhi[:, :])

            for g, tiles in enumerate(groups):
                ncg = sum(nch[j] for j in tiles)
                nlog = sum(nclo[j] for j in tiles)
                nhig = sum(nchi[j] for j in tiles)
                gs = gatp.tile([P, ncg, ROWW], bf16, tag="gs")
                # gather calls per arena for the whole group, split at GMAX
                # slots per call (1024-idx HW limit)
                def gcalls(d0, nsl, table, idx_sb, s0):
                    for q0 in range(0, nsl, GMAX):
                        q1 = min(q0 + GMAX, nsl)
                        nidx = (q1 - q0) * P
                        nc.gpsimd.dma_gather(
                            out_ap=gs[:, d0 + q0:d0 + q1, :], in_ap=table,
                            idxs_ap=idx_sb[:, (s0 + q0) * 8:(s0 + q1) * 8],
                            num_idxs=nidx, num_idxs_reg=nidx, elem_size=ROWW)

                gcalls(0, nlog, xsj[0:SPLIT, :], ixlo_sb, lo_base[tiles[0]])
                if nhig:
                    gcalls(nlog, nhig, xsj[SPLIT:NPAD, :], ixhi_sb,
                           hi_base[tiles[0]])

                gbase = grp_base[g]

                def chunk_slot(j, k):
                    """slab position + global chunk id of chunk k of tile j."""
                    if k < nclo[j]:
                        cid = glo_off[j] + k
                    else:
                        cid = ghi_off[j] + (k - nclo[j])
                    return cid - gbase, cid

                for j in tiles:
                    ncj = nch[j]
                    oh = ohp.tile([P, ncj, P], bf16, tag="oh")
                    ohT = ohp.tile([P, ncj, P], bf16, tag="ohT")
                    ex = smp.tile([P, ncj, HEADS], bf16, tag="ex")
                    den_ps = psden.tile([P, HEADS], f32, space="PSUM",
                                        tag="den")
                    # ---- phase A: scores + denominator ----
                    # sub-batches must not span the lo/hi arena boundary
                    # (slab positions and chunk ids jump there)
                    bounds = []
                    for a0, a1 in ((0, nclo[j]), (nclo[j], ncj)):
                        for b0 in range(a0, a1, B):
                            bounds.append((b0, min(b0 + B, a1)))
                    for (b0, b1) in bounds:
                        nb = b1 - b0
                        ohT_ps = psoht.tile([P, B, P], bf16, space="PSUM",
                                            tag="ohtp")
                        for k in range(b0, b1):
                            sp, cid = chunk_slot(j, k)
                            nc.vector.tensor_scalar(
                                out=oh[:, k, :], in0=iota_bf[:],
                                scalar1=toff_sb[:, cid:cid + 1], scalar2=None,
                                op0=mybir.AluOpType.is_equal)
                            nc.tensor.transpose(
                                out=ohT_ps[:, k - b0, :], in_=oh[:, k, :],
                                identity=ident_bf[:])
                        nc.scalar.copy(out=ohT[:, b0:b1, :],
                                       in_=ohT_ps[:, 0:nb, :])
                        zb = smp.tile([P, nb * HEADS], f32, tag="zb")
                        sp0, cid0 = chunk_slot(j, b0)
                        # chunks of one tile within an arena are contiguous
                        # in both the slab and the global chunk numbering
                        nc.vector.tensor_tensor(
                            out=zb[:].rearrange("p (c h) -> p c h", c=nb),
                            in0=zpre_sb[:, cid0:cid0 + nb, :],
                            in1=gs[:, sp0:sp0 + nb, HID:HID + HEADS],
                            op=mybir.AluOpType.add)
                        nc.vector.scalar_tensor_tensor(
                            out=zb[:], in0=zb[:], scalar=NEG_SLOPE,
                            op0=mybir.AluOpType.mult, in1=zb[:],
                            op1=mybir.AluOpType.max)
                        nc.scalar.activation(
                            out=ex[:, b0:b1, :].rearrange("p c h -> p (c h)"),
                            in_=zb[:], func=mybir.ActivationFunctionType.Exp)
                        for k in range(b0, b1):
                            nc.tensor.matmul(
                                out=den_ps[:], lhsT=oh[:, k, :],
                                rhs=ex[:, k, :], start=(k == 0),
                                stop=(k == ncj - 1))
                    # ---- phase B: recip ----
                    rec = smp.tile([P, HEADS], f32, tag="rec")
                    nc.vector.tensor_scalar(
                        out=rec[:], in0=den_ps[:], scalar1=1e-20, scalar2=None,
                        op0=mybir.AluOpType.max)
                    nc.vector.reciprocal(out=rec[:], in_=rec[:])
                    rec_bf = smp.tile([P, HEADS], bf16, tag="recbf")
                    nc.vector.tensor_scalar(
                        out=rec_bf[:], in0=rec[:], scalar1=1.0 / HEADS,
                        scalar2=None, op0=mybir.AluOpType.mult)
                    # ---- phase C: alpha + output ----
                    alpha = smp.tile([P, ncj], f32, tag="alpha")
                    out_ps = psout.tile([P, HID], f32, space="PSUM", tag="op")
                    for b0 in range(0, ncj, B):
                        b1 = min(b0 + B, ncj)
                        nb = b1 - b0
                        rece_ps = psrec.tile([P, B, HEADS], f32, space="PSUM",
                                             tag="rcp")
                        for k in range(b0, b1):
                            nc.tensor.matmul(
                                out=rece_ps[:, k - b0, :], lhsT=ohT[:, k, :],
                                rhs=rec_bf[:], start=True, stop=True)
                        pr = smp.tile([P, nb, HEADS], f32, tag="pr")
                        nc.vector.tensor_tensor(
                            out=pr[:], in0=ex[:, b0:b1, :],
                            in1=rece_ps[:, 0:nb, :], op=mybir.AluOpType.mult)
                        nc.vector.reduce_sum(
                            out=alpha[:, b0:b1], in_=pr[:],
                            axis=mybir.AxisListType.X)
                    for k in range(ncj):
                        sp, _ = chunk_slot(j, k)
                        nc.vector.tensor_scalar(
                            out=gs[:, sp, 0:HID], in0=gs[:, sp, 0:HID],
                            scalar1=alpha[:, k:k + 1], scalar2=None,
                            op0=mybir.AluOpType.mult)
                        nc.tensor.matmul(
                            out=out_ps[:], lhsT=oh[:, k, :],
                            rhs=gs[:, sp, 0:HID], start=(k == 0),
                            stop=(k == ncj - 1))
                    ot = smp.tile([P, HID], f32, tag="ot")
                    nc.scalar.copy(out=ot[:], in_=out_ps[:])
                    nc.sync.dma_start(out=out_sl[j * P:(j + 1) * P, :],
                                      in_=ot[:])

    nc.compile()
    return nc


def _prep(x, Wi, Wj, edge_index):
    """Host: score tables, packed gather table, per-core slot arrays."""
    x = np.asarray(x, np.float32)
    si_n = x @ np.asarray(Wi, np.float32).T      # [N, H]
    sj_n = x @ np.asarray(Wj, np.float32).T

    xsj = np.zeros((NPAD, ROWW), dtype=BF16)
    xsj[:N_NODES, 0:HID] = x.astype(BF16)
    xsj[:N_NODES, HID:HID + HEADS] = sj_n.astype(BF16)

    src = np.asarray(edge_index[0], np.int64)
    tgt = np.asarray(edge_index[1], np.int64)
    core = tgt // NLOC
    tloc = tgt % NLOC
    tile = tloc // P
    toff = tloc % P
    hi = (src >= SPLIT).astype(np.int64)

    counts = np.zeros((NCORES, NT, 2), np.int64)
    np.add.at(counts, (core, tile, hi), 1)
    nclo = [max(int(np.ceil(counts[:, j, 0].max() / P)), 1) for j in range(NT)]
    nchi = [int(np.ceil(counts[:, j, 1].max() / P)) for j in range(NT)]

    ngrp = (NT + GT - 1) // GT
    groups = [list(range(g * GT, min(g * GT + GT, NT))) for g in range(ngrp)]
    glo_off = {}
    ghi_off = {}
    cid = 0
    for tiles in groups:
        for j in tiles:
            glo_off[j] = cid
            cid += nclo[j]
        for j in tiles:
            ghi_off[j] = cid
            cid += nchi[j]
    nchunks = cid
    lo_base = np.cumsum([0] + nclo)
    hi_base = np.cumsum([0] + nchi)
    nslot_lo = int(lo_base[-1]) * P
    nslot_hi = max(int(hi_base[-1]) * P, 16)

    order = np.lexsort((hi, tile, core))
    src_s, tile_s, toff_s, hi_s, core_s, tgt_s = (
        src[order], tile[order], toff[order], hi[order], core[order],
        tgt[order])
    cuts = np.searchsorted(core_s, np.arange(NCORES + 1))

    def wrap16(a):
        if len(a) == 0:
            return np.zeros((P, 1), np.int16)
        w = a.reshape(-1, 16).T
        return np.tile(w, (8, 1)).astype(np.int16)

    per_core = []
    for c in range(NCORES):
        s, e = cuts[c], cuts[c + 1]
        csrc, ctile, ctoff, chi, ctgt = (src_s[s:e], tile_s[s:e],
                                         toff_s[s:e], hi_s[s:e], tgt_s[s:e])
        ilo = np.zeros(nslot_lo, np.int16)
        ihi = np.zeros(nslot_hi, np.int16)
        tof = np.full(nchunks * P, 999.0, np.float32)
        zpre = np.zeros((nchunks * P, HEADS), np.float32)
        key = ctile * 2 + chi
        kcuts = np.searchsorted(key, np.arange(2 * NT + 1))
        for j in range(NT):
            for a in (0, 1):
                js, je = kcuts[2 * j + a], kcuts[2 * j + a + 1]
                n = je - js
                if n == 0:
                    continue
                if a == 0:
                    ilo[lo_base[j] * P:lo_base[j] * P + n] = \
                        csrc[js:je].astype(np.int16)
                    cb = glo_off[j] * P
                else:
                    ihi[hi_base[j] * P:hi_base[j] * P + n] = \
                        (csrc[js:je] - SPLIT).astype(np.int16)
                    cb = ghi_off[j] * P
                tof[cb:cb + n] = ctoff[js:je]
                zpre[cb:cb + n] = si_n[ctgt[js:je]]
        per_core.append({
            "idxlo": wrap16(ilo),
            "idxhi": wrap16(ihi),
            "toffin": np.ascontiguousarray(
                tof.reshape(nchunks, P).T).astype(np.float32),
            "zprein": np.ascontiguousarray(
                zpre.reshape(nchunks, P, HEADS).transpose(1, 0, 2)
                .reshape(P, nchunks * HEADS)).astype(BF16),
            "xsj": xsj,
        })
    return nclo, nchi, per_core


def _get_program(nclo, nchi):
    key = (tuple(nclo), tuple(nchi))
    if key not in _CACHE:
        _CACHE.clear()
        _CACHE[key] = _build_program(nclo, nchi)
    return _CACHE[key]


def build_program_for_sim(inputs):
    nclo, nchi, _ = _prep(inputs["x"], inputs["Wi"], inputs["Wj"],
                          np.asarray(inputs["edge_index"]))
    return _get_program(nclo, nchi)


def kernel(x, Wi, Wj, edge_index):
    nclo, nchi, per_core = _prep(x, Wi, Wj, np.asarray(edge_index))
    nc = _get_program(nclo, nchi)
    res = bass_utils.run_bass_kernel_spmd(nc, per_core,
                                          core_ids=list(range(NCORES)))
    out = np.concatenate([res.results[c]["out_sl"] for c in range(NCORES)],
                         axis=0)
    return np.ascontiguousarray(out[:N_NODES]).astype(np.float32)


# revision 14
# speedup vs baseline: 2.0016x; 1.0969x over previous
"""GAT message-passing kernel for Trainium2 (8 NeuronCores, SPMD).

Target-sharded edge processing, one packed dma_gather per edge:
  - host packs [x | sj] bf16 into 512B rows of one gather table; per-edge
    si[tgt] is uploaded as a contiguous slab (chunk layout), so each edge
    costs exactly ONE 512B gather descriptor (vs 4 in the old design).
  - per-chunk one-hot (DVE is_equal, bf16/4x) drives three matmuls:
    den[t,h] += oh^T ex, rec_edge = ohT^T rec, out[t,:] += oh^T (alpha*x).
    ohT comes from a PE transpose + Act-engine PSUM->SBUF copy.
  - leaky_relu/exp/softmax-normalize all on device (DVE stt + Act exp).
  - all heavy matmuls in bf16 (1 cy/row vs 4 for f32).
No collectives: each core owns a contiguous target range and all edges
pointing into it.
"""
import numpy as np
import ml_dtypes

import concourse.mybir as mybir
from concourse import bacc, bass_utils
from concourse.tile import TileContext

BF16 = ml_dtypes.bfloat16

P = 128
NCORES = 8
N_NODES = 50000
N_EDGES = 800000
HID = 128
HEADS = 8
NPAD = 50176              # 8 * 6272
NLOC = NPAD // NCORES     # 6272 targets per core
NT = NLOC // P            # 49 tiles per core
SPLIT = 32768             # lo/hi arena split (int16 gather indices)
NEG_SLOPE = 0.01
ROWW = 256                # packed row width in bf16 elems (512B)
GT = 3                    # tiles per gather group
GMAX = 8                  # slots per dma_gather call (1024-idx HW limit)
B = 16                    # chunks per ohT/exp sub-batch

_CACHE = {}


def _build_program(nclo, nchi):
    f32 = mybir.dt.float32
    bf16 = mybir.dt.bfloat16
    nch = [a + b for a, b in zip(nclo, nchi)]
    ngrp = (NT + GT - 1) // GT
    groups = [list(range(g * GT, min(g * GT + GT, NT))) for g in range(ngrp)]
    # global chunk ids: per group: [lo(j0) lo(j1) .. | hi(j0) hi(j1) ..]
    glo_off = {}
    ghi_off = {}
    grp_base = []
    cid = 0
    for g, tiles in enumerate(groups):
        grp_base.append(cid)
        for j in tiles:
            glo_off[j] = cid
            cid += nclo[j]
        for j in tiles:
            ghi_off[j] = cid
            cid += nchi[j]
    nchunks = cid
    lo_base = np.cumsum([0] + nclo).tolist()   # slot base in lo arena
    hi_base = np.cumsum([0] + nchi).tolist()

    nslot_lo = sum(nclo) * P
    nslot_hi = max(sum(nchi) * P, 16)

    nc = bacc.Bacc("TRN2", num_devices=NCORES)

    xsj = nc.dram_tensor("xsj", [NPAD, ROWW], bf16, kind="ExternalInput")
    idxlo = nc.dram_tensor("idxlo", [P, nslot_lo // 16], mybir.dt.int16,
                           kind="ExternalInput")
    idxhi = nc.dram_tensor("idxhi", [P, nslot_hi // 16], mybir.dt.int16,
                           kind="ExternalInput")
    toffin = nc.dram_tensor("toffin", [P, nchunks], f32, kind="ExternalInput")
    zprein = nc.dram_tensor("zprein", [P, nchunks * HEADS], bf16,
                            kind="ExternalInput")
    out_sl = nc.dram_tensor("out_sl", [NLOC, HID], f32, kind="ExternalOutput")

    iota_d = nc.inline_tensor(
        np.tile(np.arange(P, dtype=BF16), (P, 1)), name="iotac")
    ident_d = nc.inline_tensor(np.eye(P, dtype=BF16), name="identc")

    with TileContext(nc) as tc:
        lp = nc.allow_low_precision(reason="bf16 one-hot transpose, no accum")
        lp.__enter__()
        with tc.tile_pool(name="const", bufs=1) as constp, \
             tc.tile_pool(name="gat", bufs=2) as gatp, \
             tc.tile_pool(name="oh", bufs=2) as ohp, \
             tc.tile_pool(name="sm", bufs=2) as smp, \
             tc.tile_pool(name="ps_oht", bufs=1, space="PSUM") as psoht, \
             tc.tile_pool(name="ps_den", bufs=2, space="PSUM") as psden, \
             tc.tile_pool(name="ps_rec", bufs=2, space="PSUM") as psrec, \
             tc.tile_pool(name="ps_out", bufs=2, space="PSUM") as psout:

            iota_bf = constp.tile([P, P], bf16)
            nc.sync.dma_start(out=iota_bf[:], in_=iota_d[:, :])
            ident_bf = constp.tile([P, P], bf16)
            nc.sync.dma_start(out=ident_bf[:], in_=ident_d[:, :])
            toff_sb = constp.tile([P, nchunks], f32)
            nc.sync.dma_start(out=toff_sb[:], in_=toffin[:, :])
            zpre_sb = constp.tile([P, nchunks, HEADS], bf16)
            nc.sync.dma_start(
                out=zpre_sb[:].rearrange("p c h -> p (c h)"),
                in_=zprein[:, :])
            ixlo_sb = constp.tile([P, nslot_lo // 16], mybir.dt.int16)
            nc.sync.dma_start(out=ixlo_sb[:], in_=idxlo[:, :])
            ixhi_sb = constp.tile([P, nslot_hi // 16], mybir.dt.int16)
            nc.sync.dma_start(out=ixhi_sb[:], in_=idxhi[:, :])

            def emit_C(j, ncj, oh, ohT, ex, rec_bf, gs, slots):
                alpha = smp.tile([P, ncj], f32, tag="alpha")
                out_ps = psout.tile([P, HID], f32, space="PSUM", tag="op")
                for b0 in range(0, ncj, B):
                    b1 = min(b0 + B, ncj)
                    nb = b1 - b0
                    rece_ps = psrec.tile([P, B, HEADS], f32, space="PSUM",
                                         tag="rcp")
                    for k in range(b0, b1):
                        nc.tensor.matmul(
                            out=rece_ps[:, k - b0, :], lhsT=ohT[:, k, :],
                            rhs=rec_bf[:], start=True, stop=True)
                    pr = smp.tile([P, nb, HEADS], f32, tag="pr")
                    nc.vector.tensor_tensor(
                        out=pr[:], in0=ex[:, b0:b1, :],
                        in1=rece_ps[:, 0:nb, :], op=mybir.AluOpType.mult)
                    nc.vector.reduce_sum(
                        out=alpha[:, b0:b1], in_=pr[:],
                        axis=mybir.AxisListType.X)
                for k in range(ncj):
                    sp = slots[k][0]
                    if k % 4 == 3:
                        # offload a quarter of the scales to the Act engine
                        nc.scalar.activation(
                            out=gs[:, sp, 0:HID], in_=gs[:, sp, 0:HID],
                            func=mybir.ActivationFunctionType.Copy,
                            scale=alpha[:, k:k + 1])
                    else:
                        nc.vector.tensor_scalar(
                            out=gs[:, sp, 0:HID], in0=gs[:, sp, 0:HID],
                            scalar1=alpha[:, k:k + 1], scalar2=None,
                            op0=mybir.AluOpType.mult)
                    nc.tensor.matmul(
                        out=out_ps[:], lhsT=oh[:, k, :],
                        rhs=gs[:, sp, 0:HID], start=(k == 0),
                        stop=(k == ncj - 1))
                ot = smp.tile([P, HID], f32, tag="ot")
                nc.scalar.copy(out=ot[:], in_=out_ps[:])
                nc.sync.dma_start(out=out_sl[j * P:(j + 1) * P, :],
                                  in_=ot[:])

            pend = None
            for g, tiles in enumerate(groups):
                ncg = sum(nch[j] for j in tiles)
                nlog = sum(nclo[j] for j in tiles)
                nhig = sum(nchi[j] for j in tiles)
                gs = gatp.tile([P, ncg, ROWW], bf16, tag="gs")
                # gather calls per arena for the whole group, split at GMAX
                # slots per call (1024-idx HW limit)
                def gcalls(d0, nsl, table, idx_sb, s0):
                    for q0 in range(0, nsl, GMAX):
                        q1 = min(q0 + GMAX, nsl)
                        nidx = (q1 - q0) * P
                        nc.gpsimd.dma_gather(
                            out_ap=gs[:, d0 + q0:d0 + q1, :], in_ap=table,
                            idxs_ap=idx_sb[:, (s0 + q0) * 8:(s0 + q1) * 8],
                            num_idxs=nidx, num_idxs_reg=nidx, elem_size=ROWW)

                gcalls(0, nlog, xsj[0:SPLIT, :], ixlo_sb, lo_base[tiles[0]])
                if nhig:
                    gcalls(nlog, nhig, xsj[SPLIT:NPAD, :], ixhi_sb,
                           hi_base[tiles[0]])

                gbase = grp_base[g]

                def chunk_slot(j, k, gbase=gbase):
                    """slab position + global chunk id of chunk k of tile j."""
                    if k < nclo[j]:
                        cid = glo_off[j] + k
                    else:
                        cid = ghi_off[j] + (k - nclo[j])
                    return cid - gbase, cid

                for j in tiles:
                    ncj = nch[j]
                    slots = [chunk_slot(j, k) for k in range(ncj)]
                    oh = ohp.tile([P, ncj, P], bf16, tag="oh")
                    ohT = ohp.tile([P, ncj, P], bf16, tag="ohT")
                    ex = smp.tile([P, ncj, HEADS], bf16, tag="ex")
                    den_ps = psden.tile([P, HEADS], f32, space="PSUM",
                                        tag="den")
                    # ---- phase A: scores + denominator ----
                    # sub-batches must not span the lo/hi arena boundary
                    # (slab positions and chunk ids jump there)
                    bounds = []
                    for a0, a1 in ((0, nclo[j]), (nclo[j], ncj)):
                        for b0 in range(a0, a1, B):
                            bounds.append((b0, min(b0 + B, a1)))
                    for (b0, b1) in bounds:
                        nb = b1 - b0
                        ohT_ps = psoht.tile([P, B, P], bf16, space="PSUM",
                                            tag="ohtp")
                        for k in range(b0, b1):
                            sp, cid = chunk_slot(j, k)
                            nc.vector.tensor_scalar(
                                out=oh[:, k, :], in0=iota_bf[:],
                                scalar1=toff_sb[:, cid:cid + 1], scalar2=None,
                                op0=mybir.AluOpType.is_equal)
                            nc.tensor.transpose(
                                out=ohT_ps[:, k - b0, :], in_=oh[:, k, :],
                                identity=ident_bf[:])
                        nc.scalar.copy(out=ohT[:, b0:b1, :],
                                       in_=ohT_ps[:, 0:nb, :])
                        zb = smp.tile([P, nb * HEADS], bf16, tag="zb")
                        sp0, cid0 = chunk_slot(j, b0)
                        # chunks of one tile within an arena are contiguous
                        # in both the slab and the global chunk numbering
                        nc.vector.tensor_tensor(
                            out=zb[:].rearrange("p (c h) -> p c h", c=nb),
                            in0=zpre_sb[:, cid0:cid0 + nb, :],
                            in1=gs[:, sp0:sp0 + nb, HID:HID + HEADS],
                            op=mybir.AluOpType.add)
                        nc.vector.scalar_tensor_tensor(
                            out=zb[:], in0=zb[:], scalar=NEG_SLOPE,
                            op0=mybir.AluOpType.mult, in1=zb[:],
                            op1=mybir.AluOpType.max)
                        nc.scalar.activation(
                            out=ex[:, b0:b1, :].rearrange("p c h -> p (c h)"),
                            in_=zb[:], func=mybir.ActivationFunctionType.Exp)
                        for k in range(b0, b1):
                            nc.tensor.matmul(
                                out=den_ps[:], lhsT=oh[:, k, :],
                                rhs=ex[:, k, :], start=(k == 0),
                                stop=(k == ncj - 1))
                    # ---- phase B: recip ----
                    rec = smp.tile([P, HEADS], f32, tag="rec")
                    nc.vector.tensor_scalar(
                        out=rec[:], in0=den_ps[:], scalar1=1e-20, scalar2=None,
                        op0=mybir.AluOpType.max)
                    nc.vector.reciprocal(out=rec[:], in_=rec[:])
                    rec_bf = smp.tile([P, HEADS], bf16, tag="recbf")
                    nc.vector.tensor_scalar(
                        out=rec_bf[:], in0=rec[:], scalar1=1.0 / HEADS,
                        scalar2=None, op0=mybir.AluOpType.mult)
                    # ---- phase C (software-pipelined by one tile) ----
                    if pend is not None:
                        emit_C(*pend)
                    pend = (j, ncj, oh, ohT, ex, rec_bf, gs, slots)

            if pend is not None:
                emit_C(*pend)

    nc.compile()
    return nc


def _prep(x, Wi, Wj, edge_index):
    """Host: score tables, packed gather table, per-core slot arrays."""
    x = np.asarray(x, np.float32)
    si_n = x @ np.asarray(Wi, np.float32).T      # [N, H]
    sj_n = x @ np.asarray(Wj, np.float32).T

    xsj = np.zeros((NPAD, ROWW), dtype=BF16)
    xsj[:N_NODES, 0:HID] = x.astype(BF16)
    xsj[:N_NODES, HID:HID + HEADS] = sj_n.astype(BF16)

    src = np.asarray(edge_index[0], np.int64)
    tgt = np.asarray(edge_index[1], np.int64)
    core = tgt // NLOC
    tloc = tgt % NLOC
    tile = tloc // P
    toff = tloc % P
    hi = (src >= SPLIT).astype(np.int64)

    counts = np.zeros((NCORES, NT, 2), np.int64)
    np.add.at(counts, (core, tile, hi), 1)
    nclo = [max(int(np.ceil(counts[:, j, 0].max() / P)), 1) for j in range(NT)]
    nchi = [int(np.ceil(counts[:, j, 1].max() / P)) for j in range(NT)]

    ngrp = (NT + GT - 1) // GT
    groups = [list(range(g * GT, min(g * GT + GT, NT))) for g in range(ngrp)]
    glo_off = {}
    ghi_off = {}
    cid = 0
    for tiles in groups:
        for j in tiles:
            glo_off[j] = cid
            cid += nclo[j]
        for j in tiles:
            ghi_off[j] = cid
            cid += nchi[j]
    nchunks = cid
    lo_base = np.cumsum([0] + nclo)
    hi_base = np.cumsum([0] + nchi)
    nslot_lo = int(lo_base[-1]) * P
    nslot_hi = max(int(hi_base[-1]) * P, 16)

    order = np.lexsort((hi, tile, core))
    src_s, tile_s, toff_s, hi_s, core_s, tgt_s = (
        src[order], tile[order], toff[order], hi[order], core[order],
        tgt[order])
    cuts = np.searchsorted(core_s, np.arange(NCORES + 1))

    def wrap16(a):
        if len(a) == 0:
            return np.zeros((P, 1), np.int16)
        w = a.reshape(-1, 16).T
        return np.tile(w, (8, 1)).astype(np.int16)

    per_core = []
    for c in range(NCORES):
        s, e = cuts[c], cuts[c + 1]
        csrc, ctile, ctoff, chi, ctgt = (src_s[s:e], tile_s[s:e],
                                         toff_s[s:e], hi_s[s:e], tgt_s[s:e])
        ilo = np.zeros(nslot_lo, np.int16)
        ihi = np.zeros(nslot_hi, np.int16)
        tof = np.full(nchunks * P, 999.0, np.float32)
        zpre = np.zeros((nchunks * P, HEADS), np.float32)
        key = ctile * 2 + chi
        kcuts = np.searchsorted(key, np.arange(2 * NT + 1))
        for j in range(NT):
            for a in (0, 1):
                js, je = kcuts[2 * j + a], kcuts[2 * j + a + 1]
                n = je - js
                if n == 0:
                    continue
                if a == 0:
                    ilo[lo_base[j] * P:lo_base[j] * P + n] = \
                        csrc[js:je].astype(np.int16)
                    cb = glo_off[j] * P
                else:
                    ihi[hi_base[j] * P:hi_base[j] * P + n] = \
                        (csrc[js:je] - SPLIT).astype(np.int16)
                    cb = ghi_off[j] * P
                tof[cb:cb + n] = ctoff[js:je]
                zpre[cb:cb + n] = si_n[ctgt[js:je]]
        per_core.append({
            "idxlo": wrap16(ilo),
            "idxhi": wrap16(ihi),
            "toffin": np.ascontiguousarray(
                tof.reshape(nchunks, P).T).astype(np.float32),
            "zprein": np.ascontiguousarray(
                zpre.reshape(nchunks, P, HEADS).transpose(1, 0, 2)
                .reshape(P, nchunks * HEADS)).astype(BF16),
            "xsj": xsj,
        })
    return nclo, nchi, per_core


def _get_program(nclo, nchi):
    key = (tuple(nclo), tuple(nchi))
    if key not in _CACHE:
        _CACHE.clear()
        _CACHE[key] = _build_program(nclo, nchi)
    return _CACHE[key]


def build_program_for_sim(inputs):
    nclo, nchi, _ = _prep(inputs["x"], inputs["Wi"], inputs["Wj"],
                          np.asarray(inputs["edge_index"]))
    return _get_program(nclo, nchi)


def kernel(x, Wi, Wj, edge_index):
    nclo, nchi, per_core = _prep(x, Wi, Wj, np.asarray(edge_index))
    nc = _get_program(nclo, nchi)
    res = bass_utils.run_bass_kernel_spmd(nc, per_core,
                                          core_ids=list(range(NCORES)))
    out = np.concatenate([res.results[c]["out_sl"] for c in range(NCORES)],
                         axis=0)
    return np.ascontiguousarray(out[:N_NODES]).astype(np.float32)
